# revision 51
# baseline (speedup 1.0000x reference)
"""Trainium2 Bass kernel for nn_BrainInspiredRouter.

Math (reference, seq_len==1 attention => attn collapses to the V path):
    attended = x @ (out_proj_w @ Wv).T + (out_proj_w @ bv + out_proj_b)
    h        = relu(attended @ W1[r].T + b1[r])          per route r
    route    = h @ W2[r].T + b2[r]
    gate     = softmax(x @ Wg.T + bg)
    out      = sum_r gate[:, r] * route[:, r, :]

Host-side constant folding (weights only, no activations):
    W1f[r]  = W1[r] @ (out_proj_w @ Wv)      -> h = relu(x @ W1f.T + b1f)
    b1f[r]  = W1[r] @ (out_proj_w@bv + out_proj_b) + b1[r]
    W2cat   = W2.transpose(0,2,1).reshape(R*DH, DOUT)
    out     = ((gate*h_flat) @ W2cat + gate @ b2) * (1/S)   (gate = exp unnorm)

Device (per core, batch-sharded 8 ways, 2048 rows each; feature-major "T"
layout so both GEMMs chain without transposes):
    gate: logitsT[8,b] -> E=exp(+bg) bf16 -> S=1@E -> rec=1/S bf16
          E/rec round-trip DRAM for 128-partition replicating reads
    per 512-col chunk:
      GEMM1: psum[h,b] = sum_k w1[k,h-tile] x xT[k,b]     (bf16 MMs)
      evict: ACT relu(+b1f) -> bf16 tmp; DVE tmp*E_bcast -> bf16 Hg
      GEMM2: psum[o,b] = sum_k2 w2[k2,o-tile] x Hg[k2,b] + b2 x E
      evict: DVE psum*rec_bcast -> bf16 -> DMA outT (host upconverts f32)

DMA discipline (the Sync engine issues triggers in order at ~600ns each,
but one trigger's packets fan out across all 16 DMA engines at ~350GB/s):
use FEW, BIG, host-contiguous transfers, ordered by first consumption:
x chunks are 1 trigger each, w1 streams as 4 chained 2MB waves matching
GEMM1(0)'s ht order, w2 loads once (resident, saves 24MB of re-reads).
"""

import numpy as np

B, D, DOUT, R = 16384, 1024, 1024, 8
DH = D // 2            # 512
RH = R * DH            # 4096
NCORES = 8
BS = B // NCORES       # 2048 rows per core
CHUNK = 512
NCHUNK = BS // CHUNK   # 4
KT = D // 128          # 8 k-tiles over D
HT = RH // 128         # 32 h-tiles
K2T = RH // 128        # 32 k-tiles over RH
OT = DOUT // 128       # 8 out-tiles
GRP = DH // 128        # 4 h-tiles per route
WB = [0, 8, 16, 24, 32]  # w1 wave boundaries in h-tiles
NW1 = len(WB) - 1

_NC_CACHE = {}


def _build_nc(mm_dt_name="bfloat16"):
    from contextlib import ExitStack

    import concourse.bass as bass
    import concourse.mybir as mybir
    import concourse.tile as tile
    from concourse import bacc

    mm_dt = getattr(mybir.dt, mm_dt_name)
    f32 = mybir.dt.float32
    AF = mybir.ActivationFunctionType

    nc = bacc.Bacc("TRN2", target_bir_lowering=False, debug=False,
                   num_devices=NCORES)

    xT = nc.dram_tensor("xT", [NCHUNK, 128, KT * CHUNK], mm_dt,
                        kind="ExternalInput")
    w1w = nc.dram_tensor("w1w", [128, KT * RH], mm_dt,
                         kind="ExternalInput")
    b1v = nc.dram_tensor("b1v", [128, HT], f32, kind="ExternalInput")
    w2 = nc.dram_tensor("w2", [OT, 128, RH], mm_dt, kind="ExternalInput")
    b2d = nc.dram_tensor("b2d", [R, DOUT], mm_dt, kind="ExternalInput")
    wgt = nc.dram_tensor("wgt", [128, KT * R], mm_dt, kind="ExternalInput")
    bgd = nc.dram_tensor("bgd", [R, 1], f32, kind="ExternalInput")
    # [ot, c, p, j] so each out tile is one dense 128KB DRAM block
    # (partition lines strided by BS hit DRAM bank conflicts at ~15GB/s)
    outT = nc.dram_tensor("outT", [OT, NCHUNK, 128, CHUNK], mm_dt,
                          kind="ExternalOutput")
    gate_scr = nc.dram_tensor("gate_scr", [R + 1, BS], mm_dt)

    with tile.TileContext(nc) as tc, ExitStack() as ctx:
        const = ctx.enter_context(tc.tile_pool(name="const", bufs=1))

        # small consts first so the gate phase isn't stuck behind bulk DMA
        bg_sb = const.tile([R, 1], f32, tag="bg")
        nc.sync.dma_start(bg_sb[:], bgd[:, :])
        ones8b = const.tile([R, 1], mm_dt, tag="ones8b")
        nc.any.memset(ones8b[:], 1.0)

        wg_all = const.tile([128, KT * R], mm_dt, tag="wg_all")
        nc.sync.dma_start(wg_all[:], wgt[:, :])
        wg_sb = [wg_all[:, k * R:(k + 1) * R] for k in range(KT)]
        b1_sb = const.tile([128, HT], f32, tag="b1")
        nc.sync.dma_start(b1_sb[:], b1v[:, :])

        xp = ctx.enter_context(tc.tile_pool(name="xp", bufs=2))
        gm = ctx.enter_context(tc.tile_pool(name="gm", bufs=2))
        # recf is consumed by the bf16 convert within the same emit
        rfp = ctx.enter_context(tc.tile_pool(name="rfp", bufs=1))
        gbcp = ctx.enter_context(tc.tile_pool(name="gbcp", bufs=2))
        hgp = ctx.enter_context(tc.tile_pool(name="hgp", bufs=1))
        tmpp = ctx.enter_context(tc.tile_pool(name="tmpp", bufs=3))
        outp = ctx.enter_context(tc.tile_pool(name="outp", bufs=2))
        p1 = ctx.enter_context(tc.tile_pool(name="p1", bufs=4, space="PSUM"))
        p2 = ctx.enter_context(tc.tile_pool(name="p2", bufs=2, space="PSUM"))
        pbc = ctx.enter_context(tc.tile_pool(name="pbc", bufs=2, space="PSUM"))

        xtiles = {}
        xdmas = {}

        def emit_x_prefetch(c, split=False):
            """whole chunk in one host-contiguous transfer (two halves for
            chunk 0 so the gate logits start on the first half sooner)"""
            xt = xp.tile([128, KT * CHUNK], mm_dt, tag="xall", name=f"x_{c}")
            half = KT * CHUNK // 2
            if split:
                nc.sync.dma_start(xt[:, 0:half], xT[c, :, 0:half])
                xdmas[c] = nc.sync.dma_start(xt[:, half:], xT[c, :, half:])
            else:
                xdmas[c] = nc.sync.dma_start(xt[:], xT[c, :, :])
            xtiles[c] = [xt[:, k * CHUNK:(k + 1) * CHUNK] for k in range(KT)]

        E9s = {}
        recs = {}
        grows = {}
        gballs = {}

        def emit_gate_logits(c):
            """E = exp(x@Wg.T + bg) for chunk c (bf16, unnorm)."""
            pg = pbc.tile([R, CHUNK], f32, tag="pb", name=f"pg_{c}")
            for k in range(KT):
                nc.tensor.matmul(pg[:], wg_sb[k][:], xtiles[c][k][:],
                                 start=(k == 0), stop=(k == KT - 1))
            E = gm.tile([R, CHUNK], mm_dt, tag="E", name=f"E_{c}")
            nc.scalar.activation(E[:], pg[:], AF.Exp, bias=bg_sb[:])
            E9s[c] = E

        def emit_gate_sum(c):
            """S = sum_r E -> rec = 1/S (bf16)."""
            ps = pbc.tile([1, CHUNK], f32, tag="pb", name=f"ps_{c}")
            nc.tensor.matmul(ps[:], ones8b[:], E9s[c][:],
                             start=True, stop=True)
            recf = rfp.tile([1, CHUNK], f32, tag="recf", name=f"recf_{c}")
            nc.vector.reciprocal_approx_fast(recf[:], ps[:])
            # rec(c)'s last read (the c bcast) precedes rec(c+1)'s write
            rec = rfp.tile([1, CHUNK], mm_dt, tag="rec", name=f"rec_{c}")
            with nc.allow_low_precision(reason="gate 1/S in bf16 is plenty"):
                nc.scalar.activation(rec[:], recf[:], AF.Copy)
            recs[c] = rec

        def emit_gate_bcast(c):
            """9 gate rows -> 128 partitions: two scratch writes + one
            replicating read PER ROW (separate triggers keep different
            DRAM rows in flight; a consolidated read serializes on one
            row's bank)."""
            sl = slice(c * CHUNK, (c + 1) * CHUNK)
            w1_ = nc.sync.dma_start(gate_scr[0:R, sl], E9s[c][:])
            w2_ = nc.sync.dma_start(gate_scr[R:R + 1, sl], recs[c][:])
            grows[c] = w2_
            g = gbcp.tile([128, (R + 1) * CHUNK], mm_dt, tag="gball",
                          name=f"gball_{c}")
            for r in range(R + 1):
                src = bass.AP(gate_scr, r * BS + c * CHUNK,
                              [[0, 128], [1, CHUNK]])
                dma = nc.sync.dma_start(g[:, r * CHUNK:(r + 1) * CHUNK],
                                        src)
                tile.add_dep_helper(dma.ins,
                                    (w1_ if r < R else w2_).ins,
                                    reason="gate bcast read after write")
            gballs[c] = g

        # prologue. Sync trigger order ~ arrival order: x(0); w1 waves 0-1
        # (wave1's trigger blocks Sync until wave0 completes, about when
        # the gate(0) chain resolves too); gate(0) round-trip; waves 2-3;
        # x(1) chained so it doesn't steal wave bandwidth; w2 + b2.
        emit_x_prefetch(0, split=True)
        emit_gate_logits(0)
        emit_gate_sum(0)

        w1_sb = [const.tile([128, KT * (WB[q + 1] - WB[q]) * 128], mm_dt,
                            tag=f"w1_{q}", name=f"w1sb{q}")
                 for q in range(NW1)]

        def w1_stat(k, ht):
            q = next(i for i in range(NW1) if WB[i + 1] > ht)
            wq = WB[q + 1] - WB[q]
            lo = k * (wq * 128) + (ht - WB[q]) * 128
            return w1_sb[q][:, lo:lo + 128]

        def w1_load(q):
            off = WB[q] * KT * 128
            width = KT * (WB[q + 1] - WB[q]) * 128
            return nc.sync.dma_start(w1_sb[q][:], w1w[:, off:off + width])

        waves = [w1_load(0)]
        emit_gate_bcast(0)
        for q in range(1, NW1):
            dma = w1_load(q)
            tile.add_dep_helper(dma.ins, waves[-1].ins,
                                reason=f"w1 wave {q} arrival order")
            waves.append(dma)
        w1_last = waves[-1]
        emit_x_prefetch(1)
        b2_sb = const.tile([R, DOUT], mm_dt, tag="b2")
        dma = nc.sync.dma_start(b2_sb[:], b2d[:, :])
        tile.add_dep_helper(dma.ins, waves[0].ins, reason="b2 after w1 head")
        w2_sb = []
        for ot in range(OT):
            wt = const.tile([128, RH], mm_dt, tag=f"w2_{ot}",
                            name=f"w2sb{ot}")
            dma = nc.sync.dma_start(wt[:], w2[ot, :, :])
            tile.add_dep_helper(dma.ins, w1_last.ins,
                                reason="w2 stream after w1 bulk load")
            w2_sb.append(wt)

        for c in range(NCHUNK):
            sl = slice(c * CHUNK, (c + 1) * CHUNK)
            xts = xtiles.pop(c)
            if 1 <= c < NCHUNK - 1:
                emit_x_prefetch(c + 1)

            # gate-aux emission points for chunk c+1 (scattered so
            # cross-engine latency hides behind GEMM1 groups; later for
            # c==0 because x(1) lands mid-wave-stream)
            gpos = {16: 'logits', 18: 'sum', 20: 'bcast'} \
                if c == 0 else {10: 'logits', 12: 'sum', 14: 'bcast'}
            hgs = []
            for ht in range(HT):
                ps1 = p1.tile([128, CHUNK], f32, tag="ps1")
                for k in range(KT):
                    nc.tensor.matmul(ps1[:], w1_stat(k, ht)[:], xts[k][:],
                                     start=(k == 0), stop=(k == KT - 1))
                tmp = tmpp.tile([128, CHUNK], mm_dt, tag="tmp",
                                name=f"tmp_{c}_{ht}")
                nc.scalar.activation(tmp[:], ps1[:], AF.Relu,
                                     bias=b1_sb[:, ht:ht + 1])
                hg = hgp.tile([128, CHUNK], mm_dt, tag=f"hg{ht}",
                              name=f"hg{ht}_{c}")
                r = ht // GRP
                nc.vector.tensor_mul(hg[:], tmp[:],
                                     gballs[c][:, r * CHUNK:(r + 1) * CHUNK])
                hgs.append(hg)
                if c + 1 < NCHUNK and ht in gpos:
                    op = gpos[ht]
                    if op == 'logits':
                        emit_gate_logits(c + 1)
                    elif op == 'sum':
                        emit_gate_sum(c + 1)
                    else:
                        emit_gate_bcast(c + 1)

            srec = gballs[c][:, R * CHUNK:(R + 1) * CHUNK]
            for ot in range(OT):
                ps2 = p2.tile([128, CHUNK], f32, tag="ps2")
                # small b2 matmul first: switching the stationary width
                # mid-chain hiccups the PE pipeline at the stop boundary
                nc.tensor.matmul(ps2[:], b2_sb[:, ot * 128:(ot + 1) * 128],
                                 E9s[c][:], start=True, stop=False)
                for k2 in range(K2T):
                    nc.tensor.matmul(ps2[:],
                                     w2_sb[ot][:, k2 * 128:(k2 + 1) * 128],
                                     hgs[k2][:],
                                     start=False, stop=(k2 == K2T - 1))
                osb = outp.tile([128, CHUNK], mm_dt, tag="osb")
                nc.vector.tensor_mul(osb[:], ps2[:], srec)
                # Activation HWDGE keeps these triggers off the Sync queue
                nc.scalar.dma_start(outT[ot, c, :, :], osb[:])
            del gballs[c], E9s[c], recs[c]
            grows.pop(c, None)

    nc.compile()
    return nc


def _get_nc(mm_dt_name="bfloat16"):
    if mm_dt_name not in _NC_CACHE:
        _NC_CACHE[mm_dt_name] = _build_nc(mm_dt_name)
    return _NC_CACHE[mm_dt_name]


def _prepare_in_maps(inputs, np_mm_dtype):
    x = np.asarray(inputs["x"], np.float32)
    in_proj_w = np.asarray(inputs["in_proj_w"], np.float32)
    in_proj_b = np.asarray(inputs["in_proj_b"], np.float32)
    out_proj_w = np.asarray(inputs["out_proj_w"], np.float32)
    out_proj_b = np.asarray(inputs["out_proj_b"], np.float32)
    W1 = np.asarray(inputs["W1"], np.float32)
    b1 = np.asarray(inputs["b1"], np.float32)
    W2 = np.asarray(inputs["W2"], np.float32)
    b2 = np.asarray(inputs["b2"], np.float32)
    Wg = np.asarray(inputs["Wg"], np.float32)
    bg = np.asarray(inputs["bg"], np.float32)

    Wv = in_proj_w[2 * D:]
    bv = in_proj_b[2 * D:]
    A = out_proj_w @ Wv                       # [D, D]
    ba = out_proj_w @ bv + out_proj_b         # [D]
    W1r = W1.reshape(RH, D)
    W1f = W1r @ A                             # [RH, D]
    b1f = W1r @ ba + b1.reshape(RH)           # [RH]
    W2cat = W2.transpose(0, 2, 1).reshape(RH, DOUT)

    # w1 wave-contiguous: per wave q the block is [p, k*Wq*128 + m]
    #   = W1f.T[k*128+p, WB[q]*128 + m]
    w1t = np.ascontiguousarray(W1f.T).reshape(KT, 128, RH)  # [k, p, h]
    blocks = []
    for q in range(NW1):
        blk = w1t[:, :, WB[q] * 128:WB[q + 1] * 128]        # [k, p, Wq*128]
        blocks.append(blk.transpose(1, 0, 2).reshape(128, -1))
    w1w_np = np.ascontiguousarray(np.concatenate(blocks, axis=1))
    b1v_np = np.ascontiguousarray(b1f.reshape(HT, 128).T)
    w2_np = np.ascontiguousarray(
        W2cat.reshape(K2T, 128, OT, 128).transpose(2, 1, 0, 3)
    ).reshape(OT, 128, RH)
    # [p, k*R+r] = Wg[r, k*128+p]: 128B-contiguous per partition line
    wgt_np = np.ascontiguousarray(Wg.reshape(R, KT, 128).transpose(2, 1, 0)
                                  ).reshape(128, KT * R)
    bg_np = np.ascontiguousarray(bg.reshape(R, 1))

    shared = {
        "w1w": w1w_np.astype(np_mm_dtype),  # [128, KT*RH] wave-major
        "b1v": b1v_np,
        "w2": w2_np.astype(np_mm_dtype),
        "b2d": b2.astype(np_mm_dtype),
        "wgt": wgt_np.astype(np_mm_dtype),
        "bgd": bg_np,
    }
    in_maps = []
    for c in range(NCORES):
        xs = x[c * BS:(c + 1) * BS]           # [BS, D]
        # [ch, p, k*CHUNK + j] = x[ch*CHUNK + j, k*128 + p]
        xT_np = np.ascontiguousarray(
            xs.reshape(NCHUNK, CHUNK, KT, 128).transpose(0, 3, 2, 1)
        ).reshape(NCHUNK, 128, KT * CHUNK)
        m = dict(shared)
        m["xT"] = xT_np.astype(np_mm_dtype)
        in_maps.append(m)
    return in_maps


def _run(inputs, trace=False, mm_dt_name="bfloat16"):
    import ml_dtypes
    from concourse.bass_utils import run_bass_kernel_spmd

    np_mm = ml_dtypes.bfloat16 if mm_dt_name == "bfloat16" else np.float32
    nc = _get_nc(mm_dt_name)
    in_maps = _prepare_in_maps(inputs, np_mm)
    res = run_bass_kernel_spmd(nc, in_maps, list(range(NCORES)), trace=trace)
    out = np.empty((B, DOUT), np.float32)
    for c in range(NCORES):
        o = np.asarray(res.results[c]["outT"], dtype=np.float32)
        o = o.reshape(OT, NCHUNK, 128, CHUNK).transpose(1, 3, 0, 2)
        out[c * BS:(c + 1) * BS] = o.reshape(BS, DOUT)
    return out, res


def kernel(**inputs):
    out, _ = _run(inputs, trace=False)
    return out


# revision 52
# speedup vs baseline: 1.0028x; 1.0028x over previous
"""Trainium2 Bass kernel for nn_BrainInspiredRouter.

Math (reference, seq_len==1 attention => attn collapses to the V path):
    attended = x @ (out_proj_w @ Wv).T + (out_proj_w @ bv + out_proj_b)
    h        = relu(attended @ W1[r].T + b1[r])          per route r
    route    = h @ W2[r].T + b2[r]
    gate     = softmax(x @ Wg.T + bg)
    out      = sum_r gate[:, r] * route[:, r, :]

Host-side constant folding (weights only, no activations):
    W1f[r]  = W1[r] @ (out_proj_w @ Wv)      -> h = relu(x @ W1f.T + b1f)
    b1f[r]  = W1[r] @ (out_proj_w@bv + out_proj_b) + b1[r]
    W2cat   = W2.transpose(0,2,1).reshape(R*DH, DOUT)
    out     = ((gate*h_flat) @ W2cat + gate @ b2) * (1/S)   (gate = exp unnorm)

Device (per core, batch-sharded 8 ways, 2048 rows each; feature-major "T"
layout so both GEMMs chain without transposes):
    gate: logitsT[8,b] -> E=exp(+bg) bf16 -> S=1@E -> rec=1/S bf16
          E/rec round-trip DRAM for 128-partition replicating reads
    per 512-col chunk:
      GEMM1: psum[h,b] = sum_k w1[k,h-tile] x xT[k,b]     (bf16 MMs)
      evict: ACT relu(+b1f) -> bf16 tmp; DVE tmp*E_bcast -> bf16 Hg
      GEMM2: psum[o,b] = sum_k2 w2[k2,o-tile] x Hg[k2,b] + b2 x E
      evict: DVE psum*rec_bcast -> bf16 -> DMA outT (host upconverts f32)

DMA discipline (the Sync engine issues triggers in order at ~600ns each,
but one trigger's packets fan out across all 16 DMA engines at ~350GB/s):
use FEW, BIG, host-contiguous transfers, ordered by first consumption:
x chunks are 1 trigger each, w1 streams as 4 chained 2MB waves matching
GEMM1(0)'s ht order, w2 loads once (resident, saves 24MB of re-reads).
"""

import numpy as np

B, D, DOUT, R = 16384, 1024, 1024, 8
DH = D // 2            # 512
RH = R * DH            # 4096
NCORES = 8
BS = B // NCORES       # 2048 rows per core
CHUNK = 512
NCHUNK = BS // CHUNK   # 4
KT = D // 128          # 8 k-tiles over D
HT = RH // 128         # 32 h-tiles
K2T = RH // 128        # 32 k-tiles over RH
OT = DOUT // 128       # 8 out-tiles
GRP = DH // 128        # 4 h-tiles per route
WB = [0, 8, 16, 24, 32]  # w1 wave boundaries in h-tiles
NW1 = len(WB) - 1

_NC_CACHE = {}


def _build_nc(mm_dt_name="bfloat16"):
    from contextlib import ExitStack

    import concourse.bass as bass
    import concourse.mybir as mybir
    import concourse.tile as tile
    from concourse import bacc

    mm_dt = getattr(mybir.dt, mm_dt_name)
    f32 = mybir.dt.float32
    AF = mybir.ActivationFunctionType

    nc = bacc.Bacc("TRN2", target_bir_lowering=False, debug=False,
                   num_devices=NCORES)

    xT = nc.dram_tensor("xT", [NCHUNK, 128, KT * CHUNK], mm_dt,
                        kind="ExternalInput")
    w1w = nc.dram_tensor("w1w", [128, KT * RH], mm_dt,
                         kind="ExternalInput")
    b1v = nc.dram_tensor("b1v", [128, HT], f32, kind="ExternalInput")
    w2 = nc.dram_tensor("w2", [OT, 128, RH], mm_dt, kind="ExternalInput")
    b2d = nc.dram_tensor("b2d", [R, DOUT], mm_dt, kind="ExternalInput")
    wgt = nc.dram_tensor("wgt", [128, KT * R], mm_dt, kind="ExternalInput")
    bgd = nc.dram_tensor("bgd", [R, 1], f32, kind="ExternalInput")
    # [ot, c, p, j] so each out tile is one dense 128KB DRAM block
    # (partition lines strided by BS hit DRAM bank conflicts at ~15GB/s)
    outT = nc.dram_tensor("outT", [OT, NCHUNK, 128, CHUNK], mm_dt,
                          kind="ExternalOutput")
    gate_scr = nc.dram_tensor("gate_scr", [R + 1, BS], mm_dt)

    with tile.TileContext(nc) as tc, ExitStack() as ctx:
        const = ctx.enter_context(tc.tile_pool(name="const", bufs=1))

        # small consts first so the gate phase isn't stuck behind bulk DMA
        bg_sb = const.tile([R, 1], f32, tag="bg")
        nc.sync.dma_start(bg_sb[:], bgd[:, :])
        ones8b = const.tile([R, 1], mm_dt, tag="ones8b")
        nc.any.memset(ones8b[:], 1.0)

        wg_all = const.tile([128, KT * R], mm_dt, tag="wg_all")
        nc.sync.dma_start(wg_all[:], wgt[:, :])
        wg_sb = [wg_all[:, k * R:(k + 1) * R] for k in range(KT)]
        b1_sb = const.tile([128, HT], f32, tag="b1")
        nc.sync.dma_start(b1_sb[:], b1v[:, :])

        xp = ctx.enter_context(tc.tile_pool(name="xp", bufs=2))
        gm = ctx.enter_context(tc.tile_pool(name="gm", bufs=2))
        # recf is consumed by the bf16 convert within the same emit
        rfp = ctx.enter_context(tc.tile_pool(name="rfp", bufs=1))
        gbcp = ctx.enter_context(tc.tile_pool(name="gbcp", bufs=2))
        hgp = ctx.enter_context(tc.tile_pool(name="hgp", bufs=1))
        tmpp = ctx.enter_context(tc.tile_pool(name="tmpp", bufs=3))
        outp = ctx.enter_context(tc.tile_pool(name="outp", bufs=2))
        p1 = ctx.enter_context(tc.tile_pool(name="p1", bufs=4, space="PSUM"))
        p2 = ctx.enter_context(tc.tile_pool(name="p2", bufs=2, space="PSUM"))
        pbc = ctx.enter_context(tc.tile_pool(name="pbc", bufs=2, space="PSUM"))

        xtiles = {}
        xdmas = {}

        def emit_x_prefetch(c, split=False):
            """whole chunk in one host-contiguous transfer (two halves for
            chunk 0 so the gate logits start on the first half sooner)"""
            xt = xp.tile([128, KT * CHUNK], mm_dt, tag="xall", name=f"x_{c}")
            half = KT * CHUNK // 2
            if split:
                nc.sync.dma_start(xt[:, 0:half], xT[c, :, 0:half])
                xdmas[c] = nc.sync.dma_start(xt[:, half:], xT[c, :, half:])
            else:
                xdmas[c] = nc.sync.dma_start(xt[:], xT[c, :, :])
            xtiles[c] = [xt[:, k * CHUNK:(k + 1) * CHUNK] for k in range(KT)]

        E9s = {}
        recs = {}
        grows = {}
        gballs = {}

        def emit_gate_logits(c):
            """E = exp(x@Wg.T + bg) for chunk c (bf16, unnorm)."""
            pg = pbc.tile([R, CHUNK], f32, tag="pb", name=f"pg_{c}")
            for k in range(KT):
                nc.tensor.matmul(pg[:], wg_sb[k][:], xtiles[c][k][:],
                                 start=(k == 0), stop=(k == KT - 1))
            E = gm.tile([R, CHUNK], mm_dt, tag="E", name=f"E_{c}")
            nc.scalar.activation(E[:], pg[:], AF.Exp, bias=bg_sb[:])
            E9s[c] = E

        def emit_gate_sum(c):
            """S = sum_r E -> rec = 1/S (bf16)."""
            ps = pbc.tile([1, CHUNK], f32, tag="pb", name=f"ps_{c}")
            nc.tensor.matmul(ps[:], ones8b[:], E9s[c][:],
                             start=True, stop=True)
            recf = rfp.tile([1, CHUNK], f32, tag="recf", name=f"recf_{c}")
            nc.vector.reciprocal_approx_fast(recf[:], ps[:])
            # rec(c)'s last read (the c bcast) precedes rec(c+1)'s write
            rec = rfp.tile([1, CHUNK], mm_dt, tag="rec", name=f"rec_{c}")
            with nc.allow_low_precision(reason="gate 1/S in bf16 is plenty"):
                nc.scalar.activation(rec[:], recf[:], AF.Copy)
            recs[c] = rec

        def emit_gate_bcast(c):
            """9 gate rows -> 128 partitions: two scratch writes + one
            replicating read PER ROW (separate triggers keep different
            DRAM rows in flight; a consolidated read serializes on one
            row's bank)."""
            sl = slice(c * CHUNK, (c + 1) * CHUNK)
            w1_ = nc.sync.dma_start(gate_scr[0:R, sl], E9s[c][:])
            w2_ = nc.sync.dma_start(gate_scr[R:R + 1, sl], recs[c][:])
            grows[c] = w2_
            g = gbcp.tile([128, (R + 1) * CHUNK], mm_dt, tag="gball",
                          name=f"gball_{c}")
            for r in range(R + 1):
                src = bass.AP(gate_scr, r * BS + c * CHUNK,
                              [[0, 128], [1, CHUNK]])
                dma = nc.sync.dma_start(g[:, r * CHUNK:(r + 1) * CHUNK],
                                        src)
                tile.add_dep_helper(dma.ins,
                                    (w1_ if r < R else w2_).ins,
                                    reason="gate bcast read after write")
            gballs[c] = g

        # prologue. Sync trigger order ~ arrival order: x(0); w1 waves 0-1
        # (wave1's trigger blocks Sync until wave0 completes, about when
        # the gate(0) chain resolves too); gate(0) round-trip; waves 2-3;
        # x(1) chained so it doesn't steal wave bandwidth; w2 + b2.
        emit_x_prefetch(0, split=True)
        emit_gate_logits(0)
        emit_gate_sum(0)

        w1_sb = [const.tile([128, KT * (WB[q + 1] - WB[q]) * 128], mm_dt,
                            tag=f"w1_{q}", name=f"w1sb{q}")
                 for q in range(NW1)]

        def w1_stat(k, ht):
            q = next(i for i in range(NW1) if WB[i + 1] > ht)
            wq = WB[q + 1] - WB[q]
            lo = k * (wq * 128) + (ht - WB[q]) * 128
            return w1_sb[q][:, lo:lo + 128]

        def w1_load(q):
            off = WB[q] * KT * 128
            width = KT * (WB[q + 1] - WB[q]) * 128
            return nc.sync.dma_start(w1_sb[q][:], w1w[:, off:off + width])

        waves = [w1_load(0)]
        emit_gate_bcast(0)
        for q in range(1, NW1):
            dma = w1_load(q)
            tile.add_dep_helper(dma.ins, waves[-1].ins,
                                reason=f"w1 wave {q} arrival order")
            waves.append(dma)
        w1_last = waves[-1]
        emit_x_prefetch(1)
        b2_sb = const.tile([R, DOUT], mm_dt, tag="b2")
        dma = nc.sync.dma_start(b2_sb[:], b2d[:, :])
        tile.add_dep_helper(dma.ins, waves[0].ins, reason="b2 after w1 head")
        w2_sb = []
        for ot in range(OT):
            wt = const.tile([128, RH], mm_dt, tag=f"w2_{ot}",
                            name=f"w2sb{ot}")
            dma = nc.sync.dma_start(wt[:], w2[ot, :, :])
            tile.add_dep_helper(dma.ins, w1_last.ins,
                                reason="w2 stream after w1 bulk load")
            w2_sb.append(wt)

        for c in range(NCHUNK):
            sl = slice(c * CHUNK, (c + 1) * CHUNK)
            xts = xtiles.pop(c)
            if 1 <= c < NCHUNK - 1:
                emit_x_prefetch(c + 1)

            # gate-aux emission points for chunk c+1 (scattered so
            # cross-engine latency hides behind GEMM1 groups; later for
            # c==0 because x(1) lands mid-wave-stream)
            gpos = {16: 'logits', 18: 'sum', 20: 'bcast'} \
                if c == 0 else {10: 'logits', 12: 'sum', 14: 'bcast'}
            hgs = []
            for ht in range(HT):
                ps1 = p1.tile([128, CHUNK], f32, tag="ps1")
                for k in range(KT):
                    nc.tensor.matmul(ps1[:], w1_stat(k, ht)[:], xts[k][:],
                                     start=(k == 0), stop=(k == KT - 1))
                tmp = tmpp.tile([128, CHUNK], mm_dt, tag="tmp",
                                name=f"tmp_{c}_{ht}")
                nc.scalar.activation(tmp[:], ps1[:], AF.Relu,
                                     bias=b1_sb[:, ht:ht + 1])
                hg = hgp.tile([128, CHUNK], mm_dt, tag=f"hg{ht}",
                              name=f"hg{ht}_{c}")
                r = ht // GRP
                nc.vector.tensor_mul(hg[:], tmp[:],
                                     gballs[c][:, r * CHUNK:(r + 1) * CHUNK])
                hgs.append(hg)
                if c + 1 < NCHUNK and ht in gpos:
                    op = gpos[ht]
                    if op == 'logits':
                        emit_gate_logits(c + 1)
                    elif op == 'sum':
                        emit_gate_sum(c + 1)
                    else:
                        emit_gate_bcast(c + 1)

            srec = gballs[c][:, R * CHUNK:(R + 1) * CHUNK]
            for ot in range(OT):
                ps2 = p2.tile([128, CHUNK], f32, tag="ps2")
                for k2 in range(K2T):
                    nc.tensor.matmul(ps2[:],
                                     w2_sb[ot][:, k2 * 128:(k2 + 1) * 128],
                                     hgs[k2][:],
                                     start=(k2 == 0), stop=False)
                nc.tensor.matmul(ps2[:], b2_sb[:, ot * 128:(ot + 1) * 128],
                                 E9s[c][:], start=False, stop=True)
                osb = outp.tile([128, CHUNK], mm_dt, tag="osb")
                nc.vector.tensor_mul(osb[:], ps2[:], srec)
                # Activation HWDGE keeps these triggers off the Sync queue
                nc.scalar.dma_start(outT[ot, c, :, :], osb[:])
            del gballs[c], E9s[c], recs[c]
            grows.pop(c, None)

    nc.compile()
    return nc


def _get_nc(mm_dt_name="bfloat16"):
    if mm_dt_name not in _NC_CACHE:
        _NC_CACHE[mm_dt_name] = _build_nc(mm_dt_name)
    return _NC_CACHE[mm_dt_name]


def _prepare_in_maps(inputs, np_mm_dtype):
    x = np.asarray(inputs["x"], np.float32)
    in_proj_w = np.asarray(inputs["in_proj_w"], np.float32)
    in_proj_b = np.asarray(inputs["in_proj_b"], np.float32)
    out_proj_w = np.asarray(inputs["out_proj_w"], np.float32)
    out_proj_b = np.asarray(inputs["out_proj_b"], np.float32)
    W1 = np.asarray(inputs["W1"], np.float32)
    b1 = np.asarray(inputs["b1"], np.float32)
    W2 = np.asarray(inputs["W2"], np.float32)
    b2 = np.asarray(inputs["b2"], np.float32)
    Wg = np.asarray(inputs["Wg"], np.float32)
    bg = np.asarray(inputs["bg"], np.float32)

    Wv = in_proj_w[2 * D:]
    bv = in_proj_b[2 * D:]
    A = out_proj_w @ Wv                       # [D, D]
    ba = out_proj_w @ bv + out_proj_b         # [D]
    W1r = W1.reshape(RH, D)
    W1f = W1r @ A                             # [RH, D]
    b1f = W1r @ ba + b1.reshape(RH)           # [RH]
    W2cat = W2.transpose(0, 2, 1).reshape(RH, DOUT)

    # w1 wave-contiguous: per wave q the block is [p, k*Wq*128 + m]
    #   = W1f.T[k*128+p, WB[q]*128 + m]
    w1t = np.ascontiguousarray(W1f.T).reshape(KT, 128, RH)  # [k, p, h]
    blocks = []
    for q in range(NW1):
        blk = w1t[:, :, WB[q] * 128:WB[q + 1] * 128]        # [k, p, Wq*128]
        blocks.append(blk.transpose(1, 0, 2).reshape(128, -1))
    w1w_np = np.ascontiguousarray(np.concatenate(blocks, axis=1))
    b1v_np = np.ascontiguousarray(b1f.reshape(HT, 128).T)
    w2_np = np.ascontiguousarray(
        W2cat.reshape(K2T, 128, OT, 128).transpose(2, 1, 0, 3)
    ).reshape(OT, 128, RH)
    # [p, k*R+r] = Wg[r, k*128+p]: 128B-contiguous per partition line
    wgt_np = np.ascontiguousarray(Wg.reshape(R, KT, 128).transpose(2, 1, 0)
                                  ).reshape(128, KT * R)
    bg_np = np.ascontiguousarray(bg.reshape(R, 1))

    shared = {
        "w1w": w1w_np.astype(np_mm_dtype),  # [128, KT*RH] wave-major
        "b1v": b1v_np,
        "w2": w2_np.astype(np_mm_dtype),
        "b2d": b2.astype(np_mm_dtype),
        "wgt": wgt_np.astype(np_mm_dtype),
        "bgd": bg_np,
    }
    in_maps = []
    for c in range(NCORES):
        xs = x[c * BS:(c + 1) * BS]           # [BS, D]
        # [ch, p, k*CHUNK + j] = x[ch*CHUNK + j, k*128 + p]
        xT_np = np.ascontiguousarray(
            xs.reshape(NCHUNK, CHUNK, KT, 128).transpose(0, 3, 2, 1)
        ).reshape(NCHUNK, 128, KT * CHUNK)
        m = dict(shared)
        m["xT"] = xT_np.astype(np_mm_dtype)
        in_maps.append(m)
    return in_maps


def _run(inputs, trace=False, mm_dt_name="bfloat16"):
    import ml_dtypes
    from concourse.bass_utils import run_bass_kernel_spmd

    np_mm = ml_dtypes.bfloat16 if mm_dt_name == "bfloat16" else np.float32
    nc = _get_nc(mm_dt_name)
    in_maps = _prepare_in_maps(inputs, np_mm)
    res = run_bass_kernel_spmd(nc, in_maps, list(range(NCORES)), trace=trace)
    out = np.empty((B, DOUT), np.float32)
    for c in range(NCORES):
        o = np.asarray(res.results[c]["outT"], dtype=np.float32)
        o = o.reshape(OT, NCHUNK, 128, CHUNK).transpose(1, 3, 0, 2)
        out[c * BS:(c + 1) * BS] = o.reshape(BS, DOUT)
    return out, res


def kernel(**inputs):
    out, _ = _run(inputs, trace=False)
    return out


# revision 53
# speedup vs baseline: 1.0147x; 1.0119x over previous
"""Trainium2 Bass kernel for nn_BrainInspiredRouter.

Math (reference, seq_len==1 attention => attn collapses to the V path):
    attended = x @ (out_proj_w @ Wv).T + (out_proj_w @ bv + out_proj_b)
    h        = relu(attended @ W1[r].T + b1[r])          per route r
    route    = h @ W2[r].T + b2[r]
    gate     = softmax(x @ Wg.T + bg)
    out      = sum_r gate[:, r] * route[:, r, :]

Host-side constant folding (weights only, no activations):
    W1f[r]  = W1[r] @ (out_proj_w @ Wv)      -> h = relu(x @ W1f.T + b1f)
    b1f[r]  = W1[r] @ (out_proj_w@bv + out_proj_b) + b1[r]
    W2cat   = W2.transpose(0,2,1).reshape(R*DH, DOUT)
    out     = ((gate*h_flat) @ W2cat + gate @ b2) * (1/S)   (gate = exp unnorm)

Device (per core, batch-sharded 8 ways, 2048 rows each; feature-major "T"
layout so both GEMMs chain without transposes):
    gate: logitsT[8,b] -> E=exp(+bg) bf16 -> S=1@E -> rec=1/S bf16
          E/rec round-trip DRAM for 128-partition replicating reads
    per 512-col chunk:
      GEMM1: psum[h,b] = sum_k w1[k,h-tile] x xT[k,b]     (bf16 MMs)
      evict: ACT relu(+b1f) -> bf16 tmp; DVE tmp*E_bcast -> bf16 Hg
      GEMM2: psum[o,b] = sum_k2 w2[k2,o-tile] x Hg[k2,b] + b2 x E
      evict: DVE psum*rec_bcast -> bf16 -> DMA outT (host upconverts f32)

DMA discipline (the Sync engine issues triggers in order at ~600ns each,
but one trigger's packets fan out across all 16 DMA engines at ~350GB/s):
use FEW, BIG, host-contiguous transfers, ordered by first consumption:
x chunks are 1 trigger each, w1 streams as 4 chained 2MB waves matching
GEMM1(0)'s ht order, w2 loads once (resident, saves 24MB of re-reads).
"""

import numpy as np

B, D, DOUT, R = 16384, 1024, 1024, 8
DH = D // 2            # 512
RH = R * DH            # 4096
NCORES = 8
BS = B // NCORES       # 2048 rows per core
CHUNK = 512
NCHUNK = BS // CHUNK   # 4
KT = D // 128          # 8 k-tiles over D
HT = RH // 128         # 32 h-tiles
K2T = RH // 128        # 32 k-tiles over RH
OT = DOUT // 128       # 8 out-tiles
GRP = DH // 128        # 4 h-tiles per route
WB = [0, 8, 16, 24, 32]  # w1 wave boundaries in h-tiles
NW1 = len(WB) - 1

_NC_CACHE = {}


def _build_nc(mm_dt_name="bfloat16"):
    from contextlib import ExitStack

    import concourse.bass as bass
    import concourse.mybir as mybir
    import concourse.tile as tile
    from concourse import bacc

    mm_dt = getattr(mybir.dt, mm_dt_name)
    f32 = mybir.dt.float32
    AF = mybir.ActivationFunctionType

    nc = bacc.Bacc("TRN2", target_bir_lowering=False, debug=False,
                   num_devices=NCORES)

    xT = nc.dram_tensor("xT", [NCHUNK, 128, KT * CHUNK], mm_dt,
                        kind="ExternalInput")
    w1w = nc.dram_tensor("w1w", [128, KT * RH], mm_dt,
                         kind="ExternalInput")
    b1v = nc.dram_tensor("b1v", [128, HT], f32, kind="ExternalInput")
    w2 = nc.dram_tensor("w2", [OT, 128, RH], mm_dt, kind="ExternalInput")
    b2d = nc.dram_tensor("b2d", [R, DOUT], mm_dt, kind="ExternalInput")
    wgt = nc.dram_tensor("wgt", [128, KT * R], mm_dt, kind="ExternalInput")
    bgd = nc.dram_tensor("bgd", [R, 1], f32, kind="ExternalInput")
    # [ot, c, p, j] so each out tile is one dense 128KB DRAM block
    # (partition lines strided by BS hit DRAM bank conflicts at ~15GB/s)
    outT = nc.dram_tensor("outT", [OT, NCHUNK, 128, CHUNK], mm_dt,
                          kind="ExternalOutput")
    gate_scr = nc.dram_tensor("gate_scr", [R + 1, BS], mm_dt)

    with tile.TileContext(nc) as tc, ExitStack() as ctx:
        const = ctx.enter_context(tc.tile_pool(name="const", bufs=1))

        # small consts first so the gate phase isn't stuck behind bulk DMA
        bg_sb = const.tile([R, 1], f32, tag="bg")
        nc.sync.dma_start(bg_sb[:], bgd[:, :])
        ones8b = const.tile([R, 1], mm_dt, tag="ones8b")
        nc.any.memset(ones8b[:], 1.0)

        wg_all = const.tile([128, KT * R], mm_dt, tag="wg_all")
        nc.sync.dma_start(wg_all[:], wgt[:, :])
        wg_sb = [wg_all[:, k * R:(k + 1) * R] for k in range(KT)]
        b1_sb = const.tile([128, HT], f32, tag="b1")
        nc.sync.dma_start(b1_sb[:], b1v[:, :])

        xp = ctx.enter_context(tc.tile_pool(name="xp", bufs=2))
        gm = ctx.enter_context(tc.tile_pool(name="gm", bufs=2))
        # recf is consumed by the bf16 convert within the same emit
        rfp = ctx.enter_context(tc.tile_pool(name="rfp", bufs=1))
        gbcp = ctx.enter_context(tc.tile_pool(name="gbcp", bufs=2))
        hgp = ctx.enter_context(tc.tile_pool(name="hgp", bufs=1))
        tmpp = ctx.enter_context(tc.tile_pool(name="tmpp", bufs=3))
        outp = ctx.enter_context(tc.tile_pool(name="outp", bufs=2))
        p1 = ctx.enter_context(tc.tile_pool(name="p1", bufs=4, space="PSUM"))
        p2 = ctx.enter_context(tc.tile_pool(name="p2", bufs=2, space="PSUM"))
        pbc = ctx.enter_context(tc.tile_pool(name="pbc", bufs=2, space="PSUM"))

        xtiles = {}
        xdmas = {}

        def emit_x_prefetch(c, split=False):
            """whole chunk in one host-contiguous transfer (two halves for
            chunk 0 so the gate logits start on the first half sooner)"""
            xt = xp.tile([128, KT * CHUNK], mm_dt, tag="xall", name=f"x_{c}")
            half = KT * CHUNK // 2
            if split:
                nc.sync.dma_start(xt[:, 0:half], xT[c, :, 0:half])
                xdmas[c] = nc.sync.dma_start(xt[:, half:], xT[c, :, half:])
            else:
                xdmas[c] = nc.sync.dma_start(xt[:], xT[c, :, :])
            xtiles[c] = [xt[:, k * CHUNK:(k + 1) * CHUNK] for k in range(KT)]

        E9s = {}
        recs = {}
        grows = {}
        gballs = {}

        def emit_gate_logits(c):
            """E = exp(x@Wg.T + bg) for chunk c (bf16, unnorm)."""
            pg = pbc.tile([R, CHUNK], f32, tag="pb", name=f"pg_{c}")
            for k in range(KT):
                nc.tensor.matmul(pg[:], wg_sb[k][:], xtiles[c][k][:],
                                 start=(k == 0), stop=(k == KT - 1))
            E = gm.tile([R, CHUNK], mm_dt, tag="E", name=f"E_{c}")
            nc.scalar.activation(E[:], pg[:], AF.Exp, bias=bg_sb[:])
            E9s[c] = E

        def emit_gate_sum(c):
            """S = sum_r E -> rec = 1/S (bf16)."""
            ps = pbc.tile([1, CHUNK], f32, tag="pb", name=f"ps_{c}")
            nc.tensor.matmul(ps[:], ones8b[:], E9s[c][:],
                             start=True, stop=True)
            recf = rfp.tile([1, CHUNK], f32, tag="recf", name=f"recf_{c}")
            nc.vector.reciprocal_approx_fast(recf[:], ps[:])
            # rec(c)'s last read (the c bcast) precedes rec(c+1)'s write
            rec = rfp.tile([1, CHUNK], mm_dt, tag="rec", name=f"rec_{c}")
            with nc.allow_low_precision(reason="gate 1/S in bf16 is plenty"):
                nc.scalar.activation(rec[:], recf[:], AF.Copy)
            recs[c] = rec

        def emit_gate_bcast(c):
            """9 gate rows -> 128 partitions: two scratch writes + one
            replicating read PER ROW (separate triggers keep different
            DRAM rows in flight; a consolidated read serializes on one
            row's bank)."""
            sl = slice(c * CHUNK, (c + 1) * CHUNK)
            w1_ = nc.sync.dma_start(gate_scr[0:R, sl], E9s[c][:])
            w2_ = nc.sync.dma_start(gate_scr[R:R + 1, sl], recs[c][:])
            grows[c] = w2_
            g = gbcp.tile([128, (R + 1) * CHUNK], mm_dt, tag="gball",
                          name=f"gball_{c}")
            for r in range(R + 1):
                src = bass.AP(gate_scr, r * BS + c * CHUNK,
                              [[0, 128], [1, CHUNK]])
                dma = nc.sync.dma_start(g[:, r * CHUNK:(r + 1) * CHUNK],
                                        src)
                tile.add_dep_helper(dma.ins,
                                    (w1_ if r < R else w2_).ins,
                                    reason="gate bcast read after write")
            gballs[c] = g

        # prologue. Sync trigger order ~ arrival order: x(0); w1 waves 0-1
        # (wave1's trigger blocks Sync until wave0 completes, about when
        # the gate(0) chain resolves too); gate(0) round-trip; waves 2-3;
        # x(1) chained so it doesn't steal wave bandwidth; w2 + b2.
        emit_x_prefetch(0)
        emit_gate_logits(0)
        emit_gate_sum(0)

        w1_sb = [const.tile([128, KT * (WB[q + 1] - WB[q]) * 128], mm_dt,
                            tag=f"w1_{q}", name=f"w1sb{q}")
                 for q in range(NW1)]

        def w1_stat(k, ht):
            q = next(i for i in range(NW1) if WB[i + 1] > ht)
            wq = WB[q + 1] - WB[q]
            lo = k * (wq * 128) + (ht - WB[q]) * 128
            return w1_sb[q][:, lo:lo + 128]

        def w1_load(q):
            off = WB[q] * KT * 128
            width = KT * (WB[q + 1] - WB[q]) * 128
            return nc.sync.dma_start(w1_sb[q][:], w1w[:, off:off + width])

        waves = [w1_load(0)]
        emit_gate_bcast(0)
        for q in range(1, NW1):
            dma = w1_load(q)
            tile.add_dep_helper(dma.ins, waves[-1].ins,
                                reason=f"w1 wave {q} arrival order")
            waves.append(dma)
        w1_last = waves[-1]
        emit_x_prefetch(1)
        b2_sb = const.tile([R, DOUT], mm_dt, tag="b2")
        dma = nc.sync.dma_start(b2_sb[:], b2d[:, :])
        tile.add_dep_helper(dma.ins, waves[0].ins, reason="b2 after w1 head")
        w2_sb = []
        for ot in range(OT):
            wt = const.tile([128, RH], mm_dt, tag=f"w2_{ot}",
                            name=f"w2sb{ot}")
            dma = nc.sync.dma_start(wt[:], w2[ot, :, :])
            tile.add_dep_helper(dma.ins, w1_last.ins,
                                reason="w2 stream after w1 bulk load")
            w2_sb.append(wt)

        for c in range(NCHUNK):
            sl = slice(c * CHUNK, (c + 1) * CHUNK)
            xts = xtiles.pop(c)
            if 1 <= c < NCHUNK - 1:
                emit_x_prefetch(c + 1)

            # gate-aux emission points for chunk c+1 (scattered so
            # cross-engine latency hides behind GEMM1 groups; later for
            # c==0 because x(1) lands mid-wave-stream)
            gpos = {16: 'logits', 18: 'sum', 20: 'bcast'} \
                if c == 0 else {10: 'logits', 12: 'sum', 14: 'bcast'}
            hgs = []
            for ht in range(HT):
                ps1 = p1.tile([128, CHUNK], f32, tag="ps1")
                for k in range(KT):
                    nc.tensor.matmul(ps1[:], w1_stat(k, ht)[:], xts[k][:],
                                     start=(k == 0), stop=(k == KT - 1))
                tmp = tmpp.tile([128, CHUNK], mm_dt, tag="tmp",
                                name=f"tmp_{c}_{ht}")
                nc.scalar.activation(tmp[:], ps1[:], AF.Relu,
                                     bias=b1_sb[:, ht:ht + 1])
                hg = hgp.tile([128, CHUNK], mm_dt, tag=f"hg{ht}",
                              name=f"hg{ht}_{c}")
                r = ht // GRP
                nc.vector.tensor_mul(hg[:], tmp[:],
                                     gballs[c][:, r * CHUNK:(r + 1) * CHUNK])
                hgs.append(hg)
                if c + 1 < NCHUNK and ht in gpos:
                    op = gpos[ht]
                    if op == 'logits':
                        emit_gate_logits(c + 1)
                    elif op == 'sum':
                        emit_gate_sum(c + 1)
                    else:
                        emit_gate_bcast(c + 1)

            srec = gballs[c][:, R * CHUNK:(R + 1) * CHUNK]
            for ot in range(OT):
                ps2 = p2.tile([128, CHUNK], f32, tag="ps2")
                for k2 in range(K2T):
                    nc.tensor.matmul(ps2[:],
                                     w2_sb[ot][:, k2 * 128:(k2 + 1) * 128],
                                     hgs[k2][:],
                                     start=(k2 == 0), stop=False)
                nc.tensor.matmul(ps2[:], b2_sb[:, ot * 128:(ot + 1) * 128],
                                 E9s[c][:], start=False, stop=True)
                osb = outp.tile([128, CHUNK], mm_dt, tag="osb")
                nc.vector.tensor_mul(osb[:], ps2[:], srec)
                # Activation HWDGE keeps these triggers off the Sync queue
                nc.scalar.dma_start(outT[ot, c, :, :], osb[:])
            del gballs[c], E9s[c], recs[c]
            grows.pop(c, None)

    nc.compile()
    return nc


def _get_nc(mm_dt_name="bfloat16"):
    if mm_dt_name not in _NC_CACHE:
        _NC_CACHE[mm_dt_name] = _build_nc(mm_dt_name)
    return _NC_CACHE[mm_dt_name]


def _prepare_in_maps(inputs, np_mm_dtype):
    x = np.asarray(inputs["x"], np.float32)
    in_proj_w = np.asarray(inputs["in_proj_w"], np.float32)
    in_proj_b = np.asarray(inputs["in_proj_b"], np.float32)
    out_proj_w = np.asarray(inputs["out_proj_w"], np.float32)
    out_proj_b = np.asarray(inputs["out_proj_b"], np.float32)
    W1 = np.asarray(inputs["W1"], np.float32)
    b1 = np.asarray(inputs["b1"], np.float32)
    W2 = np.asarray(inputs["W2"], np.float32)
    b2 = np.asarray(inputs["b2"], np.float32)
    Wg = np.asarray(inputs["Wg"], np.float32)
    bg = np.asarray(inputs["bg"], np.float32)

    Wv = in_proj_w[2 * D:]
    bv = in_proj_b[2 * D:]
    A = out_proj_w @ Wv                       # [D, D]
    ba = out_proj_w @ bv + out_proj_b         # [D]
    W1r = W1.reshape(RH, D)
    W1f = W1r @ A                             # [RH, D]
    b1f = W1r @ ba + b1.reshape(RH)           # [RH]
    W2cat = W2.transpose(0, 2, 1).reshape(RH, DOUT)

    # w1 wave-contiguous: per wave q the block is [p, k*Wq*128 + m]
    #   = W1f.T[k*128+p, WB[q]*128 + m]
    w1t = np.ascontiguousarray(W1f.T).reshape(KT, 128, RH)  # [k, p, h]
    blocks = []
    for q in range(NW1):
        blk = w1t[:, :, WB[q] * 128:WB[q + 1] * 128]        # [k, p, Wq*128]
        blocks.append(blk.transpose(1, 0, 2).reshape(128, -1))
    w1w_np = np.ascontiguousarray(np.concatenate(blocks, axis=1))
    b1v_np = np.ascontiguousarray(b1f.reshape(HT, 128).T)
    w2_np = np.ascontiguousarray(
        W2cat.reshape(K2T, 128, OT, 128).transpose(2, 1, 0, 3)
    ).reshape(OT, 128, RH)
    # [p, k*R+r] = Wg[r, k*128+p]: 128B-contiguous per partition line
    wgt_np = np.ascontiguousarray(Wg.reshape(R, KT, 128).transpose(2, 1, 0)
                                  ).reshape(128, KT * R)
    bg_np = np.ascontiguousarray(bg.reshape(R, 1))

    shared = {
        "w1w": w1w_np.astype(np_mm_dtype),  # [128, KT*RH] wave-major
        "b1v": b1v_np,
        "w2": w2_np.astype(np_mm_dtype),
        "b2d": b2.astype(np_mm_dtype),
        "wgt": wgt_np.astype(np_mm_dtype),
        "bgd": bg_np,
    }
    in_maps = []
    for c in range(NCORES):
        xs = x[c * BS:(c + 1) * BS]           # [BS, D]
        # [ch, p, k*CHUNK + j] = x[ch*CHUNK + j, k*128 + p]
        xT_np = np.ascontiguousarray(
            xs.reshape(NCHUNK, CHUNK, KT, 128).transpose(0, 3, 2, 1)
        ).reshape(NCHUNK, 128, KT * CHUNK)
        m = dict(shared)
        m["xT"] = xT_np.astype(np_mm_dtype)
        in_maps.append(m)
    return in_maps


def _run(inputs, trace=False, mm_dt_name="bfloat16"):
    import ml_dtypes
    from concourse.bass_utils import run_bass_kernel_spmd

    np_mm = ml_dtypes.bfloat16 if mm_dt_name == "bfloat16" else np.float32
    nc = _get_nc(mm_dt_name)
    in_maps = _prepare_in_maps(inputs, np_mm)
    res = run_bass_kernel_spmd(nc, in_maps, list(range(NCORES)), trace=trace)
    out = np.empty((B, DOUT), np.float32)
    for c in range(NCORES):
        o = np.asarray(res.results[c]["outT"], dtype=np.float32)
        o = o.reshape(OT, NCHUNK, 128, CHUNK).transpose(1, 3, 0, 2)
        out[c * BS:(c + 1) * BS] = o.reshape(BS, DOUT)
    return out, res


def kernel(**inputs):
    out, _ = _run(inputs, trace=False)
    return out
